# revision 7
# baseline (speedup 1.0000x reference)
"""Trainium2 Bass kernel for nn_Autoencoder (point-cloud GNN autoencoder).

Data-parallel over batch: 8 point clouds -> 8 NeuronCores. Each core runs the
full pipeline for one cloud. kNN top-21 runs on the vector engine with the
column index embedded in the low 11 mantissa bits of the negated distances
(one AND-move from PSUM + one OR with an iota constant), so indices fall out
of the max8 values directly. Neighbor-feature gathers use single InstDMAGather
calls (2560 rows each) instead of per-rank indirect DMAs. Direction-dependent
theta matmuls run in fp32r (4x PE throughput).
"""
import sys
sys.path.insert(0, '/opt/trn_rl_repo')

import numpy as np
import bass_rust
from concourse import bass, mybir, library_config
from concourse.tile import TileContext

B, V, NB, SUP = 8, 2048, 20, 4
NT = V // 128  # 16 point tiles per core
F32 = mybir.dt.float32
F32R = mybir.dt.float32r
I32 = mybir.dt.int32
I16 = mybir.dt.int16
AF = mybir.ActivationFunctionType
ALU = mybir.AluOpType


def _split_excess_waits(nc, max_waits=1):
    """Walrus here rejects >1 sync waits per instruction; move extras onto
    NOPs on the same engine right before it."""
    for f in nc.m.functions:
        for bb in f.blocks:
            insts = list(bb.instructions)
            out = []
            for inst in insts:
                si = getattr(inst, 'sync_info', None)
                if si is not None and si.on_wait and len(si.on_wait) > max_waits:
                    waits = list(si.on_wait)
                    move, keep = waits[:-max_waits], waits[-max_waits:]
                    for w in move:
                        eng = nc.engines[inst.engine]
                        nop = eng.nop(nofuse=True)
                        ni = nop.ins
                        for f2 in nc.m.functions:
                            for bb2 in f2.blocks:
                                if ni in bb2.instructions:
                                    bb2.instructions.remove(ni)
                        ni.sync_info = bass_rust.SyncInfo(on_wait=[w], on_update=[])
                        out.append(ni)
                    si.on_wait = keep
                out.append(inst)
            bb.instructions[:] = out


def _normalize_cols(d):
    n = np.sqrt((d.astype(np.float32) ** 2).sum(0))
    return (d / np.maximum(n, 1e-12)).astype(np.float32)


def _block_dirs(dirsn, K):
    """(3, K) normalized dirs -> block-diagonal (60, NB*K): row (r,d), col (r,k)."""
    bd = np.zeros((3 * NB, NB * K), np.float32)
    for r in range(NB):
        bd[3 * r:3 * r + 3, K * r:K * (r + 1)] = dirsn
    return bd


def build_kernel():
    nc = bass.Bass()
    src = nc.dram_tensor("source", [V, 3], F32, kind="ExternalInput")
    tf = nc.dram_tensor("target_feature", [V, 10], F32, kind="ExternalInput")
    # host-packed weight constants (bd* consumed by fp32r matmuls)
    bd0 = nc.dram_tensor("bd0", [60, NB * 64], F32R, kind="ExternalInput")
    bd1 = nc.dram_tensor("bd1", [60, NB * 128], F32R, kind="ExternalInput")
    bd2 = nc.dram_tensor("bd2", [60, NB * 64], F32R, kind="ExternalInput")
    bd3 = nc.dram_tensor("bd3", [60, NB * 12], F32R, kind="ExternalInput")
    wb1 = nc.dram_tensor("wb1", [17, 160], F32, kind="ExternalInput")   # [conv1_w; conv1_b]
    wba = nc.dram_tensor("wba", [11, 64], F32, kind="ExternalInput")    # [adain_w; adain_b]
    wbd1 = nc.dram_tensor("wbd1", [33, 80], F32, kind="ExternalInput")  # [dc1_w; dc1_b]
    wbd2 = nc.dram_tensor("wbd2", [17, 15], F32, kind="ExternalInput")  # [dc2_w; dc2_b]
    identm = nc.dram_tensor("identm", [128, 128], F32, kind="ExternalInput")
    iotam = nc.dram_tensor("iotam", [128, V], I32, kind="ExternalInput")
    e16 = nc.dram_tensor("e16", [16, 128], F32, kind="ExternalInput")
    out = nc.dram_tensor("out", [V, 3], F32, kind="ExternalOutput")
    # internal DRAM: gather tables (row-aligned for dma_gather) + idx staging
    vpad = nc.dram_tensor("vpad", [V, 64], F32)
    t_f1w = nc.dram_tensor("t_f1w", [V, 128], F32)
    t_tw = nc.dram_tensor("t_tw", [V, 64], F32)
    t_c1w = nc.dram_tensor("t_c1w", [V, 64], F32)
    stage = nc.dram_tensor("stage", [V, 20], I16)

    with TileContext(nc) as tc:
        with (
            tc.tile_pool(name="big", bufs=1) as big,       # (128,2048) topk buffer
            tc.tile_pool(name="mid", bufs=2) as mid,       # gathered / theta tiles
            tc.tile_pool(name="sml", bufs=4) as sml,
            tc.tile_pool(name="keep", bufs=1) as keep,     # persistent caches
            tc.tile_pool(name="psd", bufs=1, space="PSUM") as psd,
            tc.tile_pool(name="ps", bufs=2, space="PSUM") as ps,
            tc.tile_pool(name="ps2", bufs=1, space="PSUM") as ps2,
        ):
            nc.gpsimd.load_library(library_config.mlp)

            ident = keep.tile([128, 128], F32)
            nc.sync.dma_start(out=ident[:], in_=identm[:])
            iot = keep.tile([128, V], I32)
            nc.sync.dma_start(out=iot[:], in_=iotam[:])
            e16t = keep.tile([16, 128], F32)
            nc.sync.dma_start(out=e16t[:], in_=e16[:])

            # ---- vertsT (3, 2048) and lhsT/rhs for the distance matmul ----
            vT = keep.tile([3, V], F32)
            nc.sync.dma_start(out=vT[:], in_=bass.AP(src[:].tensor, 0, [[1, 3], [3, V]]))
            vT2 = keep.tile([3, V], F32)
            nc.vector.tensor_mul(out=vT2[:], in0=vT[:], in1=vT[:])
            ones3 = keep.tile([3, 1], F32)
            nc.vector.memset(ones3[:], 1.0)
            sqrow = keep.tile([1, V], F32)
            for j in range(4):
                sq_ps = ps.tile([1, 512], F32, tag="theta")
                nc.tensor.matmul(out=sq_ps[:], lhsT=ones3[:],
                                 rhs=vT2[:, bass.ts(j, 512)], start=True, stop=True)
                nc.scalar.copy(out=sqrow[:, bass.ts(j, 512)], in_=sq_ps[:])
            # lhsT (5, 2048) = [x;y;z; ones; sq] ; rhs (5, 2048) = [2x;2y;2z; -sq; -ones]
            lhsT = keep.tile([5, V], F32)
            rhsd = keep.tile([5, V], F32)
            onesrow = keep.tile([1, V], F32)
            nc.vector.memset(onesrow[:], 1.0)
            negones = keep.tile([1, V], F32)
            nc.vector.memset(negones[:], -1.0)
            negsq = keep.tile([1, V], F32)
            nc.vector.tensor_scalar_mul(negsq[:], sqrow[:], -1.0)
            nc.vector.tensor_copy(out=lhsT[:3, :], in_=vT[:])
            nc.sync.dma_start(out=lhsT[3:4, :], in_=onesrow[:])
            nc.sync.dma_start(out=lhsT[4:5, :], in_=sqrow[:])
            nc.vector.tensor_scalar_mul(rhsd[:3, :], vT[:], 2.0)
            nc.sync.dma_start(out=rhsd[3:4, :], in_=negsq[:])
            nc.sync.dma_start(out=rhsd[4:5, :], in_=negones[:])

            # padded verts table for the verts gather (cols 0:3 used)
            for t in range(NT):
                vp_dst = bass.AP(vpad[:].tensor, t * 128 * 64, [[64, 128], [1, 3]])
                nc.sync.dma_start(out=vp_dst, in_=src[t * 128:(t + 1) * 128, :])

            # persistent caches across passes
            X_all = keep.tile([128, NT * 160], I16)        # wrapped gather indices
            dnT_all = keep.tile([60, NT * 128], F32R)      # transposed unit directions
            f1_all = keep.tile([128, NT * 16], F32)
            f2_all = keep.tile([128, NT * 32], F32)
            t_all = keep.tile([128, NT * 32], F32)
            c1_all = keep.tile([128, NT * 16], F32)
            f2ctr = keep.tile([128, NT * 32], F32)         # conv1 center cache
            c1ctr = keep.tile([128, NT * 16], F32)         # dc1 center cache
            octr = keep.tile([128, NT * 3], F32)           # dc2 center cache
            vts = keep.tile([128, NT * 3], F32)            # verts per tile (i-major)
            s1acc = keep.tile([1, 32], F32)
            s2acc = keep.tile([1, 32], F32)
            nc.vector.memset(s1acc[:], 0.0)
            nc.vector.memset(s2acc[:], 0.0)
            ones1r = keep.tile([1, 128], F32)
            nc.vector.memset(ones1r[:], 1.0)
            ones128 = keep.tile([128, 1], F32)
            nc.vector.memset(ones128[:], 1.0)

            wb1s = keep.tile([17, 160], F32)
            nc.sync.dma_start(out=wb1s[:], in_=wb1[:])
            wbas = keep.tile([11, 64], F32)
            nc.sync.dma_start(out=wbas[:], in_=wba[:])
            wbd1s = keep.tile([33, 80], F32)
            nc.sync.dma_start(out=wbd1s[:], in_=wbd1[:])
            wbd2s = keep.tile([17, 15], F32)
            nc.sync.dma_start(out=wbd2s[:], in_=wbd2[:])
            bd0s = keep.tile([60, NB * 64], F32R)
            nc.sync.dma_start(out=bd0s[:], in_=bd0[:])
            bd1s = keep.tile([60, NB * 128], F32R)
            nc.sync.dma_start(out=bd1s[:], in_=bd1[:])
            bd2s = keep.tile([60, NB * 64], F32R)
            nc.sync.dma_start(out=bd2s[:], in_=bd2[:])
            bd3s = keep.tile([60, NB * 12], F32R)
            nc.sync.dma_start(out=bd3s[:], in_=bd3[:])

            nreg = nc.gpsimd.to_reg(NB * 128)

            def gather(t, table, C, dest):
                """dest (128, NB*C) <- table rows per X_all tile-t indices."""
                nc.gpsimd.dma_gather(
                    out_ap=dest[:].rearrange("p (r c) -> p r c", r=NB, c=C),
                    in_ap=table[:],
                    idxs_ap=X_all[:, t * 160:(t + 1) * 160],
                    num_idxs=NB * 128,
                    num_idxs_reg=nreg,
                    elem_size=C,
                    single_packet=False,
                )

            def theta_relu(t, bds, K, dest):
                """dest (128, NB*K) sbuf = relu(dnT_t.T @ block dirs) [fp32r]."""
                n = NB * K
                dT = dnT_all[:, t * 128:(t + 1) * 128]
                for j in range(0, n, 512):
                    w = min(512, n - j)
                    tp = ps.tile([128, 512], F32, tag="theta")
                    nc.tensor.matmul(out=tp[:, :w], lhsT=dT,
                                     rhs=bds[:, j:j + w], start=True, stop=True)
                    nc.scalar.activation(out=dest[:, j:j + w], in_=tp[:, :w], func=AF.Relu)

            def feat_matmul(t, fmap_ap, cin, wbs, nout):
                """feat (128, nout) = [fmap | 1] @ [w; b] for tile t."""
                ftp = ps2.tile([60, 128], F32, tag="misc")
                nc.tensor.transpose(out=ftp[:cin, :], in_=fmap_ap, identity=ident[:])
                lt = sml.tile([33, 128], F32, tag="lt")
                nc.scalar.copy(out=lt[:cin, :], in_=ftp[:cin, :])
                nc.sync.dma_start(out=lt[cin:cin + 1, :], in_=ones1r[:])
                fp = ps2.tile([128, 160], F32, tag="feat")
                nc.tensor.matmul(out=fp[:, :nout], lhsT=lt[:cin + 1, :], rhs=wbs[:],
                                 start=True, stop=True)
                return fp

            # ============ pass 0: dist + topk + X build + dn + conv0 ============
            # Software-pipelined: the gather-feeding chain of tile t+1 is
            # emitted before tile t's conv work, so the static per-engine order
            # doesn't stall PE/DVE behind the (slow) gathers.
            def p0_head(t):
                rowp = psd.tile([128, V], F32, tag="dist")
                for j in range(4):
                    nc.tensor.matmul(out=rowp[:, bass.ts(j, 512)],
                                     lhsT=lhsT[:, bass.ts(t, 128)],
                                     rhs=rhsd[:, bass.ts(j, 512)], start=True, stop=True)
                # embed column index into low 11 bits: rowe = (rowp & ~2047) | iota
                rowe = big.tile([128, V], F32, tag="rowe")
                nc.vector.tensor_scalar(out=rowe[:].bitcast(I32), in0=rowp[:].bitcast(I32),
                                        scalar1=~2047, scalar2=None, op0=ALU.bitwise_and)
                nc.vector.tensor_tensor(out=rowe[:].bitcast(I32), in0=rowe[:].bitcast(I32),
                                        in1=iot[:], op=ALU.bitwise_or)
                v24 = sml.tile([128, 24], F32, tag="v24")
                for r in range(3):
                    nc.vector.max(out=v24[:, r * 8:(r + 1) * 8], in_=rowe[:])
                    if r < 2:
                        nc.vector.match_replace(out=rowe[:], in_to_replace=v24[:, r * 8:(r + 1) * 8],
                                                in_values=rowe[:], imm_value=-3.0e38)
                t16 = sml.tile([128, 24], I16, tag="t16")
                iu = sml.tile([128, 24], I32, tag="iu")
                nc.vector.tensor_scalar(out=iu[:], in0=v24[:].bitcast(I32), scalar1=2047,
                                        scalar2=None, op0=ALU.bitwise_and)
                nc.vector.tensor_copy(out=t16[:], in_=iu[:])
                # X build: DRAM bounce -> 16-part perm -> shuffle+cast -> PE replicate
                nc.sync.dma_start(out=stage[t * 128:(t + 1) * 128, :], in_=t16[:, 1:21])
                s2b = sml.tile([16, 160], I16, tag="s2b")
                d1 = bass.AP(s2b.tensor, s2b[:].offset, [[160, 16], [20, 8], [1, 20]])
                s1 = bass.AP(stage[:].tensor, t * 128 * 20, [[20, 16], [320, 8], [1, 20]])
                nc.sync.dma_start(out=d1, in_=s1)
                x16f = sml.tile([16, 160], F32, tag="x16f")
                xo = bass.AP(x16f.tensor, x16f[:].offset, [[160, 16], [8, 20], [1, 8]])
                xi = bass.AP(s2b.tensor, s2b[:].offset, [[160, 16], [1, 20], [20, 8]])
                nc.vector.tensor_copy(out=xo, in_=xi)
                xps = ps2.tile([128, 160], F32, tag="feat")
                nc.tensor.matmul(out=xps[:], lhsT=e16t[:], rhs=x16f[:], start=True, stop=True)
                nc.scalar.copy(out=X_all[:, t * 160:(t + 1) * 160], in_=xps[:])

                # verts of this tile + gathered neighbor verts (64-padded rows)
                vt = vts[:, t * 3:(t + 1) * 3]
                nc.sync.dma_start(out=vt, in_=src[t * 128:(t + 1) * 128, :])
                vg = mid.tile([128, NB * 64], F32, tag="vg")
                gather(t, vpad, 64, vg)
                return vg

            def p0_tail(t, vg):
                # dvec, norms, dn  (vg cols 0:3 of each 64-block)
                vg3 = bass.AP(vg.tensor, vg[:].offset, [[vg[:].ap[0][0], 128], [64, NB], [1, 3]])
                dv = sml.tile([128, NB * 3], F32, tag="dv")
                vt_b = bass.AP(vts[:].tensor, vts[:].offset + t * 3,
                               [[NT * 3, 128], [0, NB], [1, 3]])
                nc.vector.tensor_tensor(out=dv[:], in0=vg3, in1=vt_b, op=ALU.subtract)
                dsq = sml.tile([128, NB * 3], F32, tag="dsq")
                nc.vector.tensor_mul(out=dsq[:], in0=dv[:], in1=dv[:])
                nsq = sml.tile([128, NB], F32, tag="nsq")
                nc.vector.tensor_reduce(
                    out=nsq[:], in_=dsq[:].rearrange("p (r d) -> p r d", r=NB, d=3),
                    axis=mybir.AxisListType.X, op=ALU.add)
                rn = sml.tile([128, NB], F32, tag="rn")
                nc.scalar.activation(out=rn[:], in_=nsq[:], func=AF.Sqrt)
                nc.vector.tensor_scalar_max(rn[:], rn[:], 1e-12)
                nc.vector.reciprocal(out=rn[:], in_=rn[:])
                dn = sml.tile([128, NB * 3], F32, tag="dn")
                rn_b = bass.AP(rn.tensor, rn[:].offset, [[rn[:].ap[0][0], 128], [1, NB], [0, 3]])
                nc.vector.tensor_tensor(out=dn[:], in0=dv[:], in1=rn_b, op=ALU.mult)
                dnp = ps2.tile([60, 128], F32, tag="misc")
                nc.tensor.transpose(out=dnp[:], in_=dn[:, :60], identity=ident[:])
                nc.scalar.copy(out=dnT_all[:, t * 128:(t + 1) * 128], in_=dnp[:])

                # conv0: theta only, K=64 -> f1
                th0 = mid.tile([128, NB * 128], F32, tag="th")
                theta_relu(t, bd0s, 64, th0)
                mx = sml.tile([128, 128], F32, tag="mx")
                nc.vector.tensor_reduce(
                    out=mx[:, :64], in_=bass.AP(th0.tensor, th0[:].offset,
                                                [[th0[:].ap[0][0], 128], [1, 64], [64, NB]]),
                    axis=mybir.AxisListType.X, op=ALU.max)
                f1t = sml.tile([128, 16], F32, tag="f1t")
                nc.vector.tensor_reduce(
                    out=f1t[:], in_=bass.AP(mx.tensor, mx[:].offset,
                                            [[mx[:].ap[0][0], 128], [1, 16], [16, 4]]),
                    axis=mybir.AxisListType.X, op=ALU.add)
                nc.scalar.activation(out=f1_all[:, t * 16:(t + 1) * 16], in_=f1t[:], func=AF.Relu)
                # conv1 feature table rows + center cache
                fp = feat_matmul(t, f1_all[:, t * 16:(t + 1) * 16], 16, wb1s[:], 160)
                nc.scalar.copy(out=f2ctr[:, t * 32:(t + 1) * 32], in_=fp[:, 0:32])
                sup = sml.tile([128, 128], F32, tag="sup")
                nc.scalar.copy(out=sup[:], in_=fp[:, 32:160])
                nc.sync.dma_start(out=t_f1w[t * 128:(t + 1) * 128, :], in_=sup[:])

            vg_prev = p0_head(0)
            for t in range(1, NT):
                vg_cur = p0_head(t)
                p0_tail(t - 1, vg_prev)
                vg_prev = vg_cur
            p0_tail(NT - 1, vg_prev)

            # ============ pass 1: conv1 -> f2, adain stats ============
            for t in range(NT):
                sg = mid.tile([128, NB * 128], F32, tag="sg")
                gather(t, t_f1w, 128, sg)
                th = mid.tile([128, NB * 128], F32, tag="th")
                theta_relu(t, bd1s, 128, th)
                nc.vector.tensor_mul(out=th[:], in0=th[:], in1=sg[:])
                mx = sml.tile([128, 128], F32, tag="mx")
                nc.vector.tensor_reduce(
                    out=mx[:], in_=bass.AP(th.tensor, th[:].offset,
                                           [[th[:].ap[0][0], 128], [1, 128], [128, NB]]),
                    axis=mybir.AxisListType.X, op=ALU.max)
                acc = sml.tile([128, 32], F32, tag="acc32")
                nc.vector.tensor_reduce(
                    out=acc[:], in_=bass.AP(mx.tensor, mx[:].offset,
                                            [[mx[:].ap[0][0], 128], [1, 32], [32, 4]]),
                    axis=mybir.AxisListType.X, op=ALU.add)
                f2t = f2_all[:, t * 32:(t + 1) * 32]
                nc.vector.tensor_add(out=acc[:], in0=acc[:], in1=f2ctr[:, t * 32:(t + 1) * 32])
                nc.scalar.activation(out=f2t, in_=acc[:], func=AF.Relu)
                # adain stats accumulation
                sp = ps2.tile([60, 128], F32, tag="misc")
                nc.tensor.matmul(out=sp[:1, 0:32], lhsT=ones128[:], rhs=f2t, start=True, stop=True)
                f2sq = sml.tile([128, 32], F32, tag="f2sq")
                nc.vector.tensor_mul(out=f2sq[:], in0=f2t, in1=f2t)
                nc.tensor.matmul(out=sp[:1, 32:64], lhsT=ones128[:], rhs=f2sq[:], start=True, stop=True)
                nc.vector.tensor_add(out=s1acc[:], in0=s1acc[:], in1=sp[:1, 0:32])
                nc.vector.tensor_add(out=s2acc[:], in0=s2acc[:], in1=sp[:1, 32:64])

            # ---- adain finalize: mean/rstd broadcast tile ----
            stat = keep.tile([1, 64], F32)
            nc.vector.tensor_scalar_mul(stat[:, 0:32], s1acc[:], 1.0 / V)
            m2 = keep.tile([1, 32], F32)
            nc.vector.tensor_mul(out=m2[:], in0=stat[:, 0:32], in1=s1acc[:])
            nc.vector.tensor_sub(out=m2[:], in0=s2acc[:], in1=m2[:])
            nc.vector.tensor_scalar_mul(m2[:], m2[:], 1.0 / (V - 1))
            nc.scalar.activation(out=m2[:], in_=m2[:], func=AF.Sqrt)
            nc.vector.tensor_scalar_add(m2[:], m2[:], 1e-8)
            nc.vector.reciprocal(out=stat[:, 32:64], in_=m2[:])
            bc_ps = ps2.tile([128, 160], F32, tag="feat")
            nc.tensor.matmul(out=bc_ps[:, :64], lhsT=ones1r[:], rhs=stat[:], start=True, stop=True)
            bc = keep.tile([128, 64], F32)
            nc.scalar.copy(out=bc[:], in_=bc_ps[:, :64])

            # ---- pass 1b: t = adain(f2), dc1 table ----
            for t in range(NT):
                tft = sml.tile([128, 10], F32, tag="tft")
                nc.sync.dma_start(out=tft[:], in_=tf[t * 128:(t + 1) * 128, :])
                hp = feat_matmul(t, tft[:], 10, wbas[:], 64)
                f2t = f2_all[:, t * 32:(t + 1) * 32]
                xn = sml.tile([128, 32], F32, tag="xn")
                nc.vector.tensor_sub(out=xn[:], in0=f2t, in1=bc[:, 0:32])
                nc.vector.tensor_mul(out=xn[:], in0=xn[:], in1=bc[:, 32:64])
                g1 = sml.tile([128, 32], F32, tag="g1")
                nc.scalar.add(out=g1[:], in_=hp[:, 0:32], add=1.0)
                nc.vector.tensor_mul(out=xn[:], in0=xn[:], in1=g1[:])
                tt = t_all[:, t * 32:(t + 1) * 32]
                nc.vector.tensor_add(out=tt, in0=xn[:], in1=hp[:, 32:64])
                fp = feat_matmul(t, tt, 32, wbd1s[:], 80)
                nc.scalar.copy(out=c1ctr[:, t * 16:(t + 1) * 16], in_=fp[:, 0:16])
                sup = sml.tile([128, 128], F32, tag="sup")
                nc.scalar.copy(out=sup[:, :64], in_=fp[:, 16:80])
                nc.sync.dma_start(out=t_tw[t * 128:(t + 1) * 128, :], in_=sup[:, :64])

            # ============ pass 2: dc1 -> c1, dc2 table ============
            for t in range(NT):
                sg = mid.tile([128, NB * 64], F32, tag="sg")
                gather(t, t_tw, 64, sg)
                th = mid.tile([128, NB * 64], F32, tag="th")
                theta_relu(t, bd2s, 64, th)
                nc.vector.tensor_mul(out=th[:], in0=th[:], in1=sg[:])
                mx = sml.tile([128, 128], F32, tag="mx")
                nc.vector.tensor_reduce(
                    out=mx[:, :64], in_=bass.AP(th.tensor, th[:].offset,
                                                [[th[:].ap[0][0], 128], [1, 64], [64, NB]]),
                    axis=mybir.AxisListType.X, op=ALU.max)
                acc = sml.tile([128, 16], F32, tag="acc16")
                nc.vector.tensor_reduce(
                    out=acc[:], in_=bass.AP(mx.tensor, mx[:].offset,
                                            [[mx[:].ap[0][0], 128], [1, 16], [16, 4]]),
                    axis=mybir.AxisListType.X, op=ALU.add)
                c1t = c1_all[:, t * 16:(t + 1) * 16]
                nc.vector.tensor_add(out=acc[:], in0=acc[:], in1=c1ctr[:, t * 16:(t + 1) * 16])
                nc.scalar.activation(out=c1t, in_=acc[:], func=AF.Relu)
                fp2 = feat_matmul(t, c1t, 16, wbd2s[:], 15)
                nc.scalar.copy(out=octr[:, t * 3:(t + 1) * 3], in_=fp2[:, 0:3])
                sup = sml.tile([128, 128], F32, tag="sup")
                nc.scalar.copy(out=sup[:, :12], in_=fp2[:, 3:15])
                tc_dst = bass.AP(t_c1w[:].tensor, t * 128 * 64, [[64, 128], [1, 12]])
                nc.sync.dma_start(out=tc_dst, in_=sup[:, :12])

            # ============ pass 3: dc2 -> sigmoid -> out ============
            for t in range(NT):
                sg = mid.tile([128, NB * 64], F32, tag="sg")
                gather(t, t_c1w, 64, sg)
                th = mid.tile([128, NB * 12], F32, tag="th3")
                theta_relu(t, bd3s, 12, th)
                sg12 = bass.AP(sg.tensor, sg[:].offset, [[sg[:].ap[0][0], 128], [64, NB], [1, 12]])
                th3 = bass.AP(th.tensor, th[:].offset, [[th[:].ap[0][0], 128], [12, NB], [1, 12]])
                nc.vector.tensor_tensor(out=th[:], in0=th3, in1=sg12, op=ALU.mult)
                mx = sml.tile([128, 12], F32, tag="mx12")
                nc.vector.tensor_reduce(
                    out=mx[:], in_=bass.AP(th.tensor, th[:].offset,
                                           [[th[:].ap[0][0], 128], [1, 12], [12, NB]]),
                    axis=mybir.AxisListType.X, op=ALU.max)
                acc = sml.tile([128, 3], F32, tag="acc3")
                nc.vector.tensor_reduce(
                    out=acc[:], in_=bass.AP(mx.tensor, mx[:].offset,
                                            [[mx[:].ap[0][0], 128], [1, 3], [3, 4]]),
                    axis=mybir.AxisListType.X, op=ALU.add)
                nc.vector.tensor_add(out=acc[:], in0=acc[:], in1=octr[:, t * 3:(t + 1) * 3])
                sig = sml.tile([128, 3], F32, tag="sig")
                nc.scalar.activation(out=sig[:], in_=acc[:], func=AF.Sigmoid)
                nc.sync.dma_start(out=out[t * 128:(t + 1) * 128, :], in_=sig[:])

    _split_excess_waits(nc)
    mybir.codegen_inst_isa_subclasses(nc)
    return nc


_NC_CACHE = None


def _consts(inputs):
    e16v = np.zeros((16, 128), np.float32)
    for p in range(128):
        e16v[p % 16, p] = 1.0
    c = {
        'bd0': _block_dirs(_normalize_cols(np.asarray(inputs['conv0_dirs'])), 64),
        'bd1': _block_dirs(_normalize_cols(np.asarray(inputs['conv1_dirs'])), 128),
        'bd2': _block_dirs(_normalize_cols(np.asarray(inputs['dc1_dirs'])), 64),
        'bd3': _block_dirs(_normalize_cols(np.asarray(inputs['dc2_dirs'])), 12),
        'wb1': np.vstack([np.asarray(inputs['conv1_w']), np.asarray(inputs['conv1_b'])[None]]).astype(np.float32),
        'wba': np.vstack([np.asarray(inputs['adain_w']), np.asarray(inputs['adain_b'])[None]]).astype(np.float32),
        'wbd1': np.vstack([np.asarray(inputs['dc1_w']), np.asarray(inputs['dc1_b'])[None]]).astype(np.float32),
        'wbd2': np.vstack([np.asarray(inputs['dc2_w']), np.asarray(inputs['dc2_b'])[None]]).astype(np.float32),
        'identm': np.eye(128, dtype=np.float32),
        'iotam': np.tile(np.arange(V, dtype=np.int32)[None], (128, 1)),
        'e16': e16v,
    }
    return {k: np.ascontiguousarray(v) for k, v in c.items()}


def kernel(**inputs):
    global _NC_CACHE
    from concourse.bass_utils import run_bass_kernel_spmd

    src = np.ascontiguousarray(np.asarray(inputs['source'], dtype=np.float32))
    tf = np.ascontiguousarray(np.asarray(inputs['target_feature'], dtype=np.float32))
    consts = _consts(inputs)
    if _NC_CACHE is None:
        _NC_CACHE = build_kernel()
    nc = _NC_CACHE
    in_maps = [dict(consts, source=src[b], target_feature=tf[b]) for b in range(B)]
    res = run_bass_kernel_spmd(nc, in_maps, list(range(B)))
    return np.stack([res.results[b]['out'] for b in range(B)]).astype(np.float32)


if __name__ == '__main__':
    inp = dict(np.load('/root/problem/dev/inputs.npz'))
    o = kernel(**inp)
    print(o.shape, o.dtype)


# revision 8
# speedup vs baseline: 1.1859x; 1.1859x over previous
"""Trainium2 Bass kernel for nn_Autoencoder (point-cloud GNN autoencoder).

Data-parallel over batch: 8 point clouds -> 8 NeuronCores. Each core runs the
full pipeline for one cloud. kNN top-21 runs on the vector engine with the
column index embedded in the low 11 mantissa bits of the negated distances
(one AND-move from PSUM + one OR with an iota constant), so indices fall out
of the max8 values directly. Neighbor-feature gathers use single InstDMAGather
calls (2560 rows each) instead of per-rank indirect DMAs. Direction-dependent
theta matmuls run in fp32r (4x PE throughput).
"""
import sys
sys.path.insert(0, '/opt/trn_rl_repo')

import numpy as np
import bass_rust
from concourse import bass, mybir, library_config
from concourse.tile import TileContext

B, V, NB, SUP = 8, 2048, 20, 4
NT = V // 128  # 16 point tiles per core
F32 = mybir.dt.float32
F32R = mybir.dt.float32r
I32 = mybir.dt.int32
I16 = mybir.dt.int16
AF = mybir.ActivationFunctionType
ALU = mybir.AluOpType


def _split_excess_waits(nc, max_waits=1):
    """Walrus here rejects >1 sync waits per instruction; move extras onto
    NOPs on the same engine right before it."""
    for f in nc.m.functions:
        for bb in f.blocks:
            insts = list(bb.instructions)
            out = []
            for inst in insts:
                si = getattr(inst, 'sync_info', None)
                if si is not None and si.on_wait and len(si.on_wait) > max_waits:
                    waits = list(si.on_wait)
                    move, keep = waits[:-max_waits], waits[-max_waits:]
                    for w in move:
                        eng = nc.engines[inst.engine]
                        nop = eng.nop(nofuse=True)
                        ni = nop.ins
                        for f2 in nc.m.functions:
                            for bb2 in f2.blocks:
                                if ni in bb2.instructions:
                                    bb2.instructions.remove(ni)
                        ni.sync_info = bass_rust.SyncInfo(on_wait=[w], on_update=[])
                        out.append(ni)
                    si.on_wait = keep
                out.append(inst)
            bb.instructions[:] = out


def _normalize_cols(d):
    n = np.sqrt((d.astype(np.float32) ** 2).sum(0))
    return (d / np.maximum(n, 1e-12)).astype(np.float32)


def _block_dirs(dirsn, K):
    """(3, K) normalized dirs -> block-diagonal (60, NB*K): row (r,d), col (r,k)."""
    bd = np.zeros((3 * NB, NB * K), np.float32)
    for r in range(NB):
        bd[3 * r:3 * r + 3, K * r:K * (r + 1)] = dirsn
    return bd


def build_kernel():
    nc = bass.Bass()
    src = nc.dram_tensor("source", [V, 3], F32, kind="ExternalInput")
    tf = nc.dram_tensor("target_feature", [V, 10], F32, kind="ExternalInput")
    # host-packed weight constants (bd* consumed by fp32r matmuls)
    bd0 = nc.dram_tensor("bd0", [60, NB * 64], F32R, kind="ExternalInput")
    bd1 = nc.dram_tensor("bd1", [60, NB * 128], F32R, kind="ExternalInput")
    bd2 = nc.dram_tensor("bd2", [60, NB * 64], F32R, kind="ExternalInput")
    bd3 = nc.dram_tensor("bd3", [60, NB * 12], F32R, kind="ExternalInput")
    wb1 = nc.dram_tensor("wb1", [17, 160], F32, kind="ExternalInput")   # [conv1_w; conv1_b]
    wba = nc.dram_tensor("wba", [11, 64], F32, kind="ExternalInput")    # [adain_w; adain_b]
    wbd1 = nc.dram_tensor("wbd1", [33, 80], F32, kind="ExternalInput")  # [dc1_w; dc1_b]
    wbd2 = nc.dram_tensor("wbd2", [17, 15], F32, kind="ExternalInput")  # [dc2_w; dc2_b]
    identm = nc.dram_tensor("identm", [128, 128], F32, kind="ExternalInput")
    iotam = nc.dram_tensor("iotam", [128, V], I32, kind="ExternalInput")
    e16 = nc.dram_tensor("e16", [16, 128], F32, kind="ExternalInput")
    out = nc.dram_tensor("out", [V, 3], F32, kind="ExternalOutput")
    # internal DRAM: gather tables (row-aligned for dma_gather) + idx staging
    vpad = nc.dram_tensor("vpad", [V, 64], F32)
    t_f1w = nc.dram_tensor("t_f1w", [V, 128], F32)
    t_tw = nc.dram_tensor("t_tw", [V, 64], F32)
    t_c1w = nc.dram_tensor("t_c1w", [V, 64], F32)
    stage = nc.dram_tensor("stage", [V, 20], I16)

    with TileContext(nc) as tc:
        with (
            tc.tile_pool(name="big", bufs=1) as big,       # (128,2048) topk buffer
            tc.tile_pool(name="mid", bufs=2) as mid,       # gathered / theta tiles
            tc.tile_pool(name="sml", bufs=4) as sml,
            tc.tile_pool(name="keep", bufs=1) as keep,     # persistent caches
            tc.tile_pool(name="psd", bufs=1, space="PSUM") as psd,
            tc.tile_pool(name="ps", bufs=2, space="PSUM") as ps,
            tc.tile_pool(name="ps2", bufs=1, space="PSUM") as ps2,
        ):
            nc.gpsimd.load_library(library_config.mlp)

            ident = keep.tile([128, 128], F32)
            nc.sync.dma_start(out=ident[:], in_=identm[:])
            iot = keep.tile([128, V], I32)
            nc.sync.dma_start(out=iot[:], in_=iotam[:])
            e16t = keep.tile([16, 128], F32)
            nc.sync.dma_start(out=e16t[:], in_=e16[:])

            # ---- vertsT (3, 2048) and lhsT/rhs for the distance matmul ----
            vT = keep.tile([3, V], F32)
            nc.sync.dma_start(out=vT[:], in_=bass.AP(src[:].tensor, 0, [[1, 3], [3, V]]))
            vT2 = keep.tile([3, V], F32)
            nc.vector.tensor_mul(out=vT2[:], in0=vT[:], in1=vT[:])
            ones3 = keep.tile([3, 1], F32)
            nc.vector.memset(ones3[:], 1.0)
            sqrow = keep.tile([1, V], F32)
            for j in range(4):
                sq_ps = ps.tile([1, 512], F32, tag="theta")
                nc.tensor.matmul(out=sq_ps[:], lhsT=ones3[:],
                                 rhs=vT2[:, bass.ts(j, 512)], start=True, stop=True)
                nc.scalar.copy(out=sqrow[:, bass.ts(j, 512)], in_=sq_ps[:])
            # lhsT (5, 2048) = [x;y;z; ones; sq] ; rhs (5, 2048) = [2x;2y;2z; -sq; -ones]
            lhsT = keep.tile([5, V], F32)
            rhsd = keep.tile([5, V], F32)
            onesrow = keep.tile([1, V], F32)
            nc.vector.memset(onesrow[:], 1.0)
            negones = keep.tile([1, V], F32)
            nc.vector.memset(negones[:], -1.0)
            negsq = keep.tile([1, V], F32)
            nc.vector.tensor_scalar_mul(negsq[:], sqrow[:], -1.0)
            nc.vector.tensor_copy(out=lhsT[:3, :], in_=vT[:])
            nc.sync.dma_start(out=lhsT[3:4, :], in_=onesrow[:])
            nc.sync.dma_start(out=lhsT[4:5, :], in_=sqrow[:])
            nc.vector.tensor_scalar_mul(rhsd[:3, :], vT[:], 2.0)
            nc.sync.dma_start(out=rhsd[3:4, :], in_=negsq[:])
            nc.sync.dma_start(out=rhsd[4:5, :], in_=negones[:])

            # padded verts table for the verts gather (cols 0:3 used)
            for t in range(NT):
                vp_dst = bass.AP(vpad[:].tensor, t * 128 * 64, [[64, 128], [1, 3]])
                nc.sync.dma_start(out=vp_dst, in_=src[t * 128:(t + 1) * 128, :])

            # persistent caches across passes
            X_all = keep.tile([128, NT * 160], I16)        # wrapped gather indices
            dnT_all = keep.tile([60, NT * 128], F32R)      # transposed unit directions
            f1_all = keep.tile([128, NT * 16], F32)
            f2_all = keep.tile([128, NT * 32], F32)
            t_all = keep.tile([128, NT * 32], F32)
            c1_all = keep.tile([128, NT * 16], F32)
            f2ctr = keep.tile([128, NT * 32], F32)         # conv1 center cache
            c1ctr = keep.tile([128, NT * 16], F32)         # dc1 center cache
            octr = keep.tile([128, NT * 3], F32)           # dc2 center cache
            vts = keep.tile([128, NT * 3], F32)            # verts per tile (i-major)
            s1acc = keep.tile([1, 32], F32)
            s2acc = keep.tile([1, 32], F32)
            nc.vector.memset(s1acc[:], 0.0)
            nc.vector.memset(s2acc[:], 0.0)
            ones1r = keep.tile([1, 128], F32)
            nc.vector.memset(ones1r[:], 1.0)
            ones128 = keep.tile([128, 1], F32)
            nc.vector.memset(ones128[:], 1.0)

            wb1s = keep.tile([17, 160], F32)
            nc.sync.dma_start(out=wb1s[:], in_=wb1[:])
            wbas = keep.tile([11, 64], F32)
            nc.sync.dma_start(out=wbas[:], in_=wba[:])
            wbd1s = keep.tile([33, 80], F32)
            nc.sync.dma_start(out=wbd1s[:], in_=wbd1[:])
            wbd2s = keep.tile([17, 15], F32)
            nc.sync.dma_start(out=wbd2s[:], in_=wbd2[:])
            bd0s = keep.tile([60, NB * 64], F32R)
            nc.sync.dma_start(out=bd0s[:], in_=bd0[:])
            bd1s = keep.tile([60, NB * 128], F32R)
            nc.sync.dma_start(out=bd1s[:], in_=bd1[:])
            bd2s = keep.tile([60, NB * 64], F32R)
            nc.sync.dma_start(out=bd2s[:], in_=bd2[:])
            bd3s = keep.tile([60, NB * 12], F32R)
            nc.sync.dma_start(out=bd3s[:], in_=bd3[:])

            nreg = nc.gpsimd.to_reg(NB * 128)

            def gather(t, table, C, dest):
                """dest (128, NB*C) <- table rows per X_all tile-t indices."""
                nc.gpsimd.dma_gather(
                    out_ap=dest[:].rearrange("p (r c) -> p r c", r=NB, c=C),
                    in_ap=table[:],
                    idxs_ap=X_all[:, t * 160:(t + 1) * 160],
                    num_idxs=NB * 128,
                    num_idxs_reg=nreg,
                    elem_size=C,
                    single_packet=False,
                )

            def theta_relu(t, bds, K, dest):
                """dest (128, NB*K) sbuf = relu(dnT_t.T @ block dirs) [fp32r]."""
                n = NB * K
                dT = dnT_all[:, t * 128:(t + 1) * 128]
                for j in range(0, n, 512):
                    w = min(512, n - j)
                    tp = ps.tile([128, 512], F32, tag="theta")
                    nc.tensor.matmul(out=tp[:, :w], lhsT=dT,
                                     rhs=bds[:, j:j + w], start=True, stop=True)
                    nc.scalar.activation(out=dest[:, j:j + w], in_=tp[:, :w], func=AF.Relu)

            def feat_matmul(t, fmap_ap, cin, wbs, nout):
                """feat (128, nout) = [fmap | 1] @ [w; b] for tile t."""
                ftp = ps2.tile([60, 128], F32, tag="misc")
                nc.tensor.transpose(out=ftp[:cin, :], in_=fmap_ap, identity=ident[:])
                lt = sml.tile([33, 128], F32, tag="lt")
                nc.scalar.copy(out=lt[:cin, :], in_=ftp[:cin, :])
                nc.sync.dma_start(out=lt[cin:cin + 1, :], in_=ones1r[:])
                fp = ps2.tile([128, 160], F32, tag="feat")
                nc.tensor.matmul(out=fp[:, :nout], lhsT=lt[:cin + 1, :], rhs=wbs[:],
                                 start=True, stop=True)
                return fp

            # ============ pass 0: dist + topk + X build + dn + conv0 ============
            # Software-pipelined: the gather-feeding chain of tile t+1 is
            # emitted before tile t's conv work, so the static per-engine order
            # doesn't stall PE/DVE behind the (slow) gathers.
            def p0_head(t):
                rowp = psd.tile([128, V], F32, tag="dist")
                for j in range(4):
                    nc.tensor.matmul(out=rowp[:, bass.ts(j, 512)],
                                     lhsT=lhsT[:, bass.ts(t, 128)],
                                     rhs=rhsd[:, bass.ts(j, 512)], start=True, stop=True)
                # embed column index into low 11 bits: rowe = (rowp & ~2047) | iota
                rowe = big.tile([128, V], F32, tag="rowe")
                nc.vector.tensor_scalar(out=rowe[:].bitcast(I32), in0=rowp[:].bitcast(I32),
                                        scalar1=~2047, scalar2=None, op0=ALU.bitwise_and)
                nc.vector.tensor_tensor(out=rowe[:].bitcast(I32), in0=rowe[:].bitcast(I32),
                                        in1=iot[:], op=ALU.bitwise_or)
                v24 = sml.tile([128, 24], F32, tag="v24")
                for r in range(3):
                    nc.vector.max(out=v24[:, r * 8:(r + 1) * 8], in_=rowe[:])
                    if r < 2:
                        nc.vector.match_replace(out=rowe[:], in_to_replace=v24[:, r * 8:(r + 1) * 8],
                                                in_values=rowe[:], imm_value=-3.0e38)
                t16 = sml.tile([128, 24], I16, tag="t16")
                iu = sml.tile([128, 24], I32, tag="iu")
                nc.vector.tensor_scalar(out=iu[:], in0=v24[:].bitcast(I32), scalar1=2047,
                                        scalar2=None, op0=ALU.bitwise_and)
                nc.vector.tensor_copy(out=t16[:], in_=iu[:])
                # X build: DRAM bounce -> 16-part perm -> shuffle+cast -> PE replicate
                nc.sync.dma_start(out=stage[t * 128:(t + 1) * 128, :], in_=t16[:, 1:21])
                s2b = sml.tile([16, 160], I16, tag="s2b")
                d1 = bass.AP(s2b.tensor, s2b[:].offset, [[160, 16], [20, 8], [1, 20]])
                s1 = bass.AP(stage[:].tensor, t * 128 * 20, [[20, 16], [320, 8], [1, 20]])
                nc.sync.dma_start(out=d1, in_=s1)
                x16f = sml.tile([16, 160], F32, tag="x16f")
                xo = bass.AP(x16f.tensor, x16f[:].offset, [[160, 16], [8, 20], [1, 8]])
                xi = bass.AP(s2b.tensor, s2b[:].offset, [[160, 16], [1, 20], [20, 8]])
                nc.vector.tensor_copy(out=xo, in_=xi)
                xps = ps.tile([128, 512], F32, tag="theta")
                nc.tensor.matmul(out=xps[:, :160], lhsT=e16t[:], rhs=x16f[:], start=True, stop=True)
                nc.scalar.copy(out=X_all[:, t * 160:(t + 1) * 160], in_=xps[:, :160])

                # verts of this tile + gathered neighbor verts (64-padded rows)
                vt = vts[:, t * 3:(t + 1) * 3]
                nc.sync.dma_start(out=vt, in_=src[t * 128:(t + 1) * 128, :])
                vg = mid.tile([128, NB * 64], F32, tag="vg")
                gather(t, vpad, 64, vg)
                return vg

            def p0_tail(t, vg):
                # dvec, norms, dn  (vg cols 0:3 of each 64-block)
                vg3 = bass.AP(vg.tensor, vg[:].offset, [[vg[:].ap[0][0], 128], [64, NB], [1, 3]])
                dv = sml.tile([128, NB * 3], F32, tag="dv")
                vt_b = bass.AP(vts[:].tensor, vts[:].offset + t * 3,
                               [[NT * 3, 128], [0, NB], [1, 3]])
                nc.vector.tensor_tensor(out=dv[:], in0=vg3, in1=vt_b, op=ALU.subtract)
                dsq = sml.tile([128, NB * 3], F32, tag="dsq")
                nc.vector.tensor_mul(out=dsq[:], in0=dv[:], in1=dv[:])
                nsq = sml.tile([128, NB], F32, tag="nsq")
                nc.vector.tensor_reduce(
                    out=nsq[:], in_=dsq[:].rearrange("p (r d) -> p r d", r=NB, d=3),
                    axis=mybir.AxisListType.X, op=ALU.add)
                rn = sml.tile([128, NB], F32, tag="rn")
                nc.scalar.activation(out=rn[:], in_=nsq[:], func=AF.Sqrt)
                nc.vector.tensor_scalar_max(rn[:], rn[:], 1e-12)
                nc.vector.reciprocal(out=rn[:], in_=rn[:])
                dn = sml.tile([128, NB * 3], F32, tag="dn")
                rn_b = bass.AP(rn.tensor, rn[:].offset, [[rn[:].ap[0][0], 128], [1, NB], [0, 3]])
                nc.vector.tensor_tensor(out=dn[:], in0=dv[:], in1=rn_b, op=ALU.mult)
                dnp = ps2.tile([60, 128], F32, tag="misc")
                nc.tensor.transpose(out=dnp[:], in_=dn[:, :60], identity=ident[:])
                nc.scalar.copy(out=dnT_all[:, t * 128:(t + 1) * 128], in_=dnp[:])

                # conv0: theta only, K=64 -> f1
                th0 = mid.tile([128, NB * 128], F32, tag="th")
                theta_relu(t, bd0s, 64, th0)
                mx = sml.tile([128, 128], F32, tag="mx")
                nc.vector.tensor_reduce(
                    out=mx[:, :64], in_=bass.AP(th0.tensor, th0[:].offset,
                                                [[th0[:].ap[0][0], 128], [1, 64], [64, NB]]),
                    axis=mybir.AxisListType.X, op=ALU.max)
                f1t = sml.tile([128, 16], F32, tag="f1t")
                nc.vector.tensor_reduce(
                    out=f1t[:], in_=bass.AP(mx.tensor, mx[:].offset,
                                            [[mx[:].ap[0][0], 128], [1, 16], [16, 4]]),
                    axis=mybir.AxisListType.X, op=ALU.add)
                nc.scalar.activation(out=f1_all[:, t * 16:(t + 1) * 16], in_=f1t[:], func=AF.Relu)
                # conv1 feature table rows + center cache
                fp = feat_matmul(t, f1_all[:, t * 16:(t + 1) * 16], 16, wb1s[:], 160)
                nc.scalar.copy(out=f2ctr[:, t * 32:(t + 1) * 32], in_=fp[:, 0:32])
                sup = sml.tile([128, 128], F32, tag="sup")
                nc.scalar.copy(out=sup[:], in_=fp[:, 32:160])
                nc.sync.dma_start(out=t_f1w[t * 128:(t + 1) * 128, :], in_=sup[:])

            vg_prev = p0_head(0)
            for t in range(1, NT):
                vg_cur = p0_head(t)
                p0_tail(t - 1, vg_prev)
                vg_prev = vg_cur
            p0_tail(NT - 1, vg_prev)

            # ============ pass 1: conv1 -> f2, adain stats ============
            for t in range(NT):
                sg = mid.tile([128, NB * 128], F32, tag="sg")
                gather(t, t_f1w, 128, sg)
                th = mid.tile([128, NB * 128], F32, tag="th")
                theta_relu(t, bd1s, 128, th)
                nc.vector.tensor_mul(out=th[:], in0=th[:], in1=sg[:])
                mx = sml.tile([128, 128], F32, tag="mx")
                nc.vector.tensor_reduce(
                    out=mx[:], in_=bass.AP(th.tensor, th[:].offset,
                                           [[th[:].ap[0][0], 128], [1, 128], [128, NB]]),
                    axis=mybir.AxisListType.X, op=ALU.max)
                acc = sml.tile([128, 32], F32, tag="acc32")
                nc.vector.tensor_reduce(
                    out=acc[:], in_=bass.AP(mx.tensor, mx[:].offset,
                                            [[mx[:].ap[0][0], 128], [1, 32], [32, 4]]),
                    axis=mybir.AxisListType.X, op=ALU.add)
                f2t = f2_all[:, t * 32:(t + 1) * 32]
                nc.vector.tensor_add(out=acc[:], in0=acc[:], in1=f2ctr[:, t * 32:(t + 1) * 32])
                nc.scalar.activation(out=f2t, in_=acc[:], func=AF.Relu)
                # adain stats accumulation
                sp = ps2.tile([60, 128], F32, tag="misc")
                nc.tensor.matmul(out=sp[:1, 0:32], lhsT=ones128[:], rhs=f2t, start=True, stop=True)
                f2sq = sml.tile([128, 32], F32, tag="f2sq")
                nc.vector.tensor_mul(out=f2sq[:], in0=f2t, in1=f2t)
                nc.tensor.matmul(out=sp[:1, 32:64], lhsT=ones128[:], rhs=f2sq[:], start=True, stop=True)
                nc.vector.tensor_add(out=s1acc[:], in0=s1acc[:], in1=sp[:1, 0:32])
                nc.vector.tensor_add(out=s2acc[:], in0=s2acc[:], in1=sp[:1, 32:64])

            # ---- adain finalize: mean/rstd broadcast tile ----
            stat = keep.tile([1, 64], F32)
            nc.vector.tensor_scalar_mul(stat[:, 0:32], s1acc[:], 1.0 / V)
            m2 = keep.tile([1, 32], F32)
            nc.vector.tensor_mul(out=m2[:], in0=stat[:, 0:32], in1=s1acc[:])
            nc.vector.tensor_sub(out=m2[:], in0=s2acc[:], in1=m2[:])
            nc.vector.tensor_scalar_mul(m2[:], m2[:], 1.0 / (V - 1))
            nc.scalar.activation(out=m2[:], in_=m2[:], func=AF.Sqrt)
            nc.vector.tensor_scalar_add(m2[:], m2[:], 1e-8)
            nc.vector.reciprocal(out=stat[:, 32:64], in_=m2[:])
            bc_ps = ps2.tile([128, 160], F32, tag="feat")
            nc.tensor.matmul(out=bc_ps[:, :64], lhsT=ones1r[:], rhs=stat[:], start=True, stop=True)
            bc = keep.tile([128, 64], F32)
            nc.scalar.copy(out=bc[:], in_=bc_ps[:, :64])

            # ---- pass 1b: t = adain(f2), dc1 table ----
            for t in range(NT):
                tft = sml.tile([128, 10], F32, tag="tft")
                nc.sync.dma_start(out=tft[:], in_=tf[t * 128:(t + 1) * 128, :])
                hp = feat_matmul(t, tft[:], 10, wbas[:], 64)
                f2t = f2_all[:, t * 32:(t + 1) * 32]
                xn = sml.tile([128, 32], F32, tag="xn")
                nc.vector.tensor_sub(out=xn[:], in0=f2t, in1=bc[:, 0:32])
                nc.vector.tensor_mul(out=xn[:], in0=xn[:], in1=bc[:, 32:64])
                g1 = sml.tile([128, 32], F32, tag="g1")
                nc.scalar.add(out=g1[:], in_=hp[:, 0:32], add=1.0)
                nc.vector.tensor_mul(out=xn[:], in0=xn[:], in1=g1[:])
                tt = t_all[:, t * 32:(t + 1) * 32]
                nc.vector.tensor_add(out=tt, in0=xn[:], in1=hp[:, 32:64])
                fp = feat_matmul(t, tt, 32, wbd1s[:], 80)
                nc.scalar.copy(out=c1ctr[:, t * 16:(t + 1) * 16], in_=fp[:, 0:16])
                sup = sml.tile([128, 128], F32, tag="sup")
                nc.scalar.copy(out=sup[:, :64], in_=fp[:, 16:80])
                nc.sync.dma_start(out=t_tw[t * 128:(t + 1) * 128, :], in_=sup[:, :64])

            # ============ pass 2: dc1 -> c1, dc2 table ============
            for t in range(NT):
                sg = mid.tile([128, NB * 64], F32, tag="sg")
                gather(t, t_tw, 64, sg)
                th = mid.tile([128, NB * 64], F32, tag="th")
                theta_relu(t, bd2s, 64, th)
                nc.vector.tensor_mul(out=th[:], in0=th[:], in1=sg[:])
                mx = sml.tile([128, 128], F32, tag="mx")
                nc.vector.tensor_reduce(
                    out=mx[:, :64], in_=bass.AP(th.tensor, th[:].offset,
                                                [[th[:].ap[0][0], 128], [1, 64], [64, NB]]),
                    axis=mybir.AxisListType.X, op=ALU.max)
                acc = sml.tile([128, 16], F32, tag="acc16")
                nc.vector.tensor_reduce(
                    out=acc[:], in_=bass.AP(mx.tensor, mx[:].offset,
                                            [[mx[:].ap[0][0], 128], [1, 16], [16, 4]]),
                    axis=mybir.AxisListType.X, op=ALU.add)
                c1t = c1_all[:, t * 16:(t + 1) * 16]
                nc.vector.tensor_add(out=acc[:], in0=acc[:], in1=c1ctr[:, t * 16:(t + 1) * 16])
                nc.scalar.activation(out=c1t, in_=acc[:], func=AF.Relu)
                fp2 = feat_matmul(t, c1t, 16, wbd2s[:], 15)
                nc.scalar.copy(out=octr[:, t * 3:(t + 1) * 3], in_=fp2[:, 0:3])
                sup = sml.tile([128, 128], F32, tag="sup")
                nc.scalar.copy(out=sup[:, :12], in_=fp2[:, 3:15])
                tc_dst = bass.AP(t_c1w[:].tensor, t * 128 * 64, [[64, 128], [1, 12]])
                nc.sync.dma_start(out=tc_dst, in_=sup[:, :12])

            # ============ pass 3: dc2 -> sigmoid -> out ============
            for t in range(NT):
                sg = mid.tile([128, NB * 64], F32, tag="sg")
                gather(t, t_c1w, 64, sg)
                th = mid.tile([128, NB * 12], F32, tag="th3")
                theta_relu(t, bd3s, 12, th)
                sg12 = bass.AP(sg.tensor, sg[:].offset, [[sg[:].ap[0][0], 128], [64, NB], [1, 12]])
                th3 = bass.AP(th.tensor, th[:].offset, [[th[:].ap[0][0], 128], [12, NB], [1, 12]])
                nc.vector.tensor_tensor(out=th[:], in0=th3, in1=sg12, op=ALU.mult)
                mx = sml.tile([128, 12], F32, tag="mx12")
                nc.vector.tensor_reduce(
                    out=mx[:], in_=bass.AP(th.tensor, th[:].offset,
                                           [[th[:].ap[0][0], 128], [1, 12], [12, NB]]),
                    axis=mybir.AxisListType.X, op=ALU.max)
                acc = sml.tile([128, 3], F32, tag="acc3")
                nc.vector.tensor_reduce(
                    out=acc[:], in_=bass.AP(mx.tensor, mx[:].offset,
                                            [[mx[:].ap[0][0], 128], [1, 3], [3, 4]]),
                    axis=mybir.AxisListType.X, op=ALU.add)
                nc.vector.tensor_add(out=acc[:], in0=acc[:], in1=octr[:, t * 3:(t + 1) * 3])
                sig = sml.tile([128, 3], F32, tag="sig")
                nc.scalar.activation(out=sig[:], in_=acc[:], func=AF.Sigmoid)
                nc.sync.dma_start(out=out[t * 128:(t + 1) * 128, :], in_=sig[:])

    _split_excess_waits(nc)
    mybir.codegen_inst_isa_subclasses(nc)
    return nc


_NC_CACHE = None


def _consts(inputs):
    e16v = np.zeros((16, 128), np.float32)
    for p in range(128):
        e16v[p % 16, p] = 1.0
    c = {
        'bd0': _block_dirs(_normalize_cols(np.asarray(inputs['conv0_dirs'])), 64),
        'bd1': _block_dirs(_normalize_cols(np.asarray(inputs['conv1_dirs'])), 128),
        'bd2': _block_dirs(_normalize_cols(np.asarray(inputs['dc1_dirs'])), 64),
        'bd3': _block_dirs(_normalize_cols(np.asarray(inputs['dc2_dirs'])), 12),
        'wb1': np.vstack([np.asarray(inputs['conv1_w']), np.asarray(inputs['conv1_b'])[None]]).astype(np.float32),
        'wba': np.vstack([np.asarray(inputs['adain_w']), np.asarray(inputs['adain_b'])[None]]).astype(np.float32),
        'wbd1': np.vstack([np.asarray(inputs['dc1_w']), np.asarray(inputs['dc1_b'])[None]]).astype(np.float32),
        'wbd2': np.vstack([np.asarray(inputs['dc2_w']), np.asarray(inputs['dc2_b'])[None]]).astype(np.float32),
        'identm': np.eye(128, dtype=np.float32),
        'iotam': np.tile(np.arange(V, dtype=np.int32)[None], (128, 1)),
        'e16': e16v,
    }
    return {k: np.ascontiguousarray(v) for k, v in c.items()}


def kernel(**inputs):
    global _NC_CACHE
    from concourse.bass_utils import run_bass_kernel_spmd

    src = np.ascontiguousarray(np.asarray(inputs['source'], dtype=np.float32))
    tf = np.ascontiguousarray(np.asarray(inputs['target_feature'], dtype=np.float32))
    consts = _consts(inputs)
    if _NC_CACHE is None:
        _NC_CACHE = build_kernel()
    nc = _NC_CACHE
    in_maps = [dict(consts, source=src[b], target_feature=tf[b]) for b in range(B)]
    res = run_bass_kernel_spmd(nc, in_maps, list(range(B)))
    return np.stack([res.results[b]['out'] for b in range(B)]).astype(np.float32)


if __name__ == '__main__':
    inp = dict(np.load('/root/problem/dev/inputs.npz'))
    o = kernel(**inp)
    print(o.shape, o.dtype)
